# revision 1
# baseline (speedup 1.0000x reference)
# Multi-head attention block (QKV proj + per-head q/k layernorm + softmax
# attention + output proj) on 8 Trainium2 NeuronCores.
#
# Sharding: data-parallel over (batch, query-half). Core c handles batch
# c//2, query tokens [ (c%2)*1024, (c%2+1)*1024 ). Each core computes K/V
# for its batch's full 2048 tokens (replicated within the batch pair), so
# there is no cross-core communication; the host just concatenates the 8
# disjoint output chunks.
#
# On-device dataflow per core:
#   xT (pre-transposed on host, bf16) -> q/k/v in natural [token, feature]
#   layout via PE matmuls (bias added with a K=1 ones-row matmul into PSUM)
#   -> q,k layernorm fused into PSUM eviction (bn_stats/bn_aggr + per-head
#   tensor_scalar) -> DMA-transpose (bf16) q,k to [feature, token] layout ->
#   per head: scores^T = k_h^T-stationary x q_h^T-moving, exp on ACT with
#   the 1/sqrt(D) scale folded in, attn@v with a ones-column appended to v
#   so the softmax normalizer Z rides along as PSUM row 64 -> normalize by
#   1/Z (broadcast via a DRAM bounce) -> output proj from the already
#   feature-major y^T tiles, bias again via K=1 matmul, f32 out.
import contextlib

import numpy as np
import ml_dtypes

B, T, E = 4, 2048, 1024
H, D = 16, 64
P = 128
EPS = 1e-5
SCALE = 0.125  # 1/sqrt(D)
TQ = T // 2          # query tokens per core
KB = E // P          # contraction blocks
MQ = TQ // P         # query token tiles
MKV = T // P         # kv token tiles
FT = E // P          # feature tiles (qT/kT/yT)
NCORES = 8
HC = 8               # heads per 512-wide feature chunk
NCH = 512

_BUILT = {}
_last_in_maps = None


def _build_real(affine: bool):
    import concourse.bass as bass
    import concourse.bacc as bacc
    import concourse.tile as tile
    from concourse import mybir

    f32 = mybir.dt.float32
    bf16 = mybir.dt.bfloat16
    AF = mybir.ActivationFunctionType
    OP = mybir.AluOpType

    nc = bacc.Bacc("TRN2", target_bir_lowering=False)
    xT_q = nc.declare_dram_parameter("xT_q", [E, TQ], bf16, isOutput=False)
    xT_kv = nc.declare_dram_parameter("xT_kv", [E, T], bf16, isOutput=False)
    Wqkv = nc.declare_dram_parameter("Wqkv", [E, 3 * E], bf16, isOutput=False)
    bqkv = nc.declare_dram_parameter("bqkv", [3 * E], bf16, isOutput=False)
    q_gamma = nc.declare_dram_parameter("q_gamma", [D], f32, isOutput=False)
    q_beta = nc.declare_dram_parameter("q_beta", [D], f32, isOutput=False)
    k_gamma = nc.declare_dram_parameter("k_gamma", [D], f32, isOutput=False)
    k_beta = nc.declare_dram_parameter("k_beta", [D], f32, isOutput=False)
    Wproj = nc.declare_dram_parameter("Wproj", [E, E], bf16, isOutput=False)
    bproj = nc.declare_dram_parameter("bproj", [E], bf16, isOutput=False)
    out = nc.declare_dram_parameter("out", [TQ, E], f32, isOutput=True)

    def bcast_dram(dst, src_ap, nparts):
        ap = bass.AP(tensor=src_ap.tensor, offset=src_ap.offset,
                     ap=[[0, nparts], *src_ap.ap])
        nc.gpsimd.dma_start(out=dst, in_=ap)

    with tile.TileContext(nc) as tc, contextlib.ExitStack() as top:
        const = top.enter_context(tc.tile_pool(name="const", bufs=1))
        yT_pool = top.enter_context(tc.tile_pool(name="yT_pool", bufs=1))
        va_pool = top.enter_context(tc.tile_pool(name="va_pool", bufs=1))
        qkT_pool = top.enter_context(tc.tile_pool(name="qkT_pool", bufs=1))
        dr = top.enter_context(tc.tile_pool(name="dr", bufs=4, space="DRAM"))
        ps = top.enter_context(tc.tile_pool(name="ps", bufs=1, space="PSUM"))

        ones = const.tile([P, P], bf16)
        nc.vector.memset(ones[:], 1.0)
        eps_t = const.tile([P, 1], f32)
        nc.vector.memset(eps_t[:], EPS)
        bqkv_row = const.tile([P, 3 * E], bf16)
        nc.sync.dma_start(out=bqkv_row[0:1, :], in_=bqkv[:])
        bproj_row = const.tile([P, E], bf16)
        nc.sync.dma_start(out=bproj_row[0:1, :], in_=bproj[:])
        wp_all = const.tile([P, KB, E], bf16)
        nc.sync.dma_start(out=wp_all[:],
                          in_=Wproj[:].rearrange("(kb p) f -> p kb f", p=P))
        if affine:
            gq_t = const.tile([P, D], bf16)
            bq_t = const.tile([P, D], bf16)
            gk_t = const.tile([P, D], bf16)
            bk_t = const.tile([P, D], bf16)
            gq_f = const.tile([P, D], f32)
            bq_f = const.tile([P, D], f32)
            bcast_dram(gq_f[:], q_gamma[:], P)
            bcast_dram(bq_f[:], q_beta[:], P)
            nc.vector.tensor_copy(out=gq_t[:], in_=gq_f[:])
            nc.vector.tensor_copy(out=bq_t[:], in_=bq_f[:])
            bcast_dram(gq_f[:], k_gamma[:], P)
            bcast_dram(bq_f[:], k_beta[:], P)
            nc.vector.tensor_copy(out=gk_t[:], in_=gq_f[:])
            nc.vector.tensor_copy(out=bk_t[:], in_=bq_f[:])

        va_all = va_pool.tile([P, MKV, H, D + 1], bf16)   # v + ones col
        qT_all = qkT_pool.tile([P, FT, TQ], bf16)
        kT_all = qkT_pool.tile([P, FT, T], bf16)
        yT_all = yT_pool.tile([P, FT, TQ], bf16)

        def ln_evict(work, pt, dst_slice, gt, bt):
            # LayerNorm over each head's D=64 slice, PSUM -> bf16 SBUF
            stats = work.tile([P, HC, 6], f32, tag="stats", bufs=3)
            mv = work.tile([P, HC, 2], f32, tag="mv", bufs=3)
            for h8 in range(HC):
                nc.vector.bn_stats(out=stats[:, h8, :],
                                   in_=pt[:, h8 * D:(h8 + 1) * D])
            for h8 in range(HC):
                nc.vector.bn_aggr(out=mv[:, h8, :], in_=stats[:, h8, :])
            std = work.tile([P, HC], f32, tag="std", bufs=3)
            nc.scalar.activation(out=std[:], in_=mv[:, :, 1],
                                 func=AF.Sqrt, bias=eps_t[:])
            rstd = work.tile([P, HC], f32, tag="rstd", bufs=3)
            nc.vector.reciprocal(out=rstd[:], in_=std[:])
            negb = work.tile([P, HC], f32, tag="negb", bufs=3)
            nc.vector.tensor_tensor(out=negb[:], in0=mv[:, :, 0],
                                    in1=rstd[:], op=OP.mult)
            nc.vector.tensor_scalar(out=negb[:], in0=negb[:],
                                    scalar1=-1.0, scalar2=None, op0=OP.mult)
            if affine:
                tmp = work.tile([P, HC, D], bf16, tag="lntmp", bufs=3)
                for h8 in range(HC):
                    nc.vector.tensor_scalar(out=tmp[:, h8, :],
                                            in0=pt[:, h8 * D:(h8 + 1) * D],
                                            scalar1=rstd[:, h8:h8 + 1],
                                            scalar2=negb[:, h8:h8 + 1],
                                            op0=OP.mult, op1=OP.add)
                gden = bass.AP(tensor=gt[:].tensor, offset=gt[:].offset,
                               ap=[gt[:].ap[0], [0, HC], [1, D]])
                bden = bass.AP(tensor=bt[:].tensor, offset=bt[:].offset,
                               ap=[bt[:].ap[0], [0, HC], [1, D]])
                nc.vector.tensor_tensor(out=tmp[:], in0=tmp[:], in1=gden,
                                        op=OP.mult)
                nc.vector.tensor_tensor(out=dst_slice, in0=tmp[:], in1=bden,
                                        op=OP.add)
            else:
                dst3 = dst_slice.rearrange("p (h d) -> p h d", h=HC)
                for h8 in range(HC):
                    nc.vector.tensor_scalar(out=dst3[:, h8, :],
                                            in0=pt[:, h8 * D:(h8 + 1) * D],
                                            scalar1=rstd[:, h8:h8 + 1],
                                            scalar2=negb[:, h8:h8 + 1],
                                            op0=OP.mult, op1=OP.add)

        # ---- phase A: QKV projections + LN / v-pack, transposes inline ----
        with contextlib.ExitStack() as pa:
            wchunks = pa.enter_context(tc.tile_pool(name="wchunks", bufs=2))
            xs = pa.enter_context(tc.tile_pool(name="xs", bufs=3))
            work = pa.enter_context(tc.tile_pool(name="work", bufs=1))
            qn_pool = pa.enter_context(tc.tile_pool(name="qn_pool", bufs=3))

            def load_wch(kind, c):
                f_base = {"q": 0, "k": E, "v": 2 * E}[kind] + c * NCH
                wch = wchunks.tile([P, KB, NCH], bf16, name=f"w_{kind}{c}",
                                   tag="wch", bufs=5)
                nc.sync.dma_start(
                    out=wch[:],
                    in_=Wqkv[:, f_base:f_base + NCH].rearrange(
                        "(kb p) f -> p kb f", p=P))
                return wch, f_base

            def qkv_mm(xm, wch, f_base, name):
                pt = ps.tile([P, NCH], f32, name=name, tag="py", bufs=2)
                for kb in range(KB):
                    nc.tensor.matmul(pt[:], xm[:, kb, :], wch[:, kb, :],
                                     start=(kb == 0), stop=False)
                nc.tensor.matmul(pt[:], ones[0:1, :],
                                 bqkv_row[0:1, f_base:f_base + NCH],
                                 start=False, stop=True)
                return pt

            def transpose_out(nt, dstT, m):
                for ft in range(FT):
                    nc.sync.dma_start(out=dstT[:, ft, m * P:(m + 1) * P],
                                      in_=nt[:, ft * P:(ft + 1) * P],
                                      transpose=True)

            # q pass: query-chunk tokens only
            wq = [load_wch("q", c) for c in range(2)]
            for m in range(MQ):
                xm = xs.tile([P, KB, P], bf16, name=f"x_q_{m}", tag="xq")
                nc.sync.dma_start(
                    out=xm[:],
                    in_=xT_q[:, m * P:(m + 1) * P].rearrange(
                        "(kb p) t -> p kb t", p=P))
                nt = qn_pool.tile([P, E], bf16, name=f"n_q_{m}", tag="nq")
                for c in range(2):
                    pt = qkv_mm(xm, wq[c][0], wq[c][1], f"pt_q{c}_{m}")
                    ln_evict(work, pt, nt[:, c * NCH:(c + 1) * NCH],
                             gq_t if affine else None,
                             bq_t if affine else None)
                transpose_out(nt, qT_all, m)

            # k+v pass: full batch tokens, one x load per tile
            wk = [load_wch("k", c) for c in range(2)]
            wv = [load_wch("v", c) for c in range(2)]
            for m in range(MKV):
                xm = xs.tile([P, KB, P], bf16, name=f"x_kv_{m}", tag="xkv")
                nc.sync.dma_start(
                    out=xm[:],
                    in_=xT_kv[:, m * P:(m + 1) * P].rearrange(
                        "(kb p) t -> p kb t", p=P))
                nt = qn_pool.tile([P, E], bf16, name=f"n_k_{m}", tag="nk")
                for c in range(2):
                    pt = qkv_mm(xm, wk[c][0], wk[c][1], f"pt_k{c}_{m}")
                    ln_evict(work, pt, nt[:, c * NCH:(c + 1) * NCH],
                             gk_t if affine else None,
                             bk_t if affine else None)
                transpose_out(nt, kT_all, m)
                nc.vector.memset(va_all[:, m, :, D], 1.0)
                for c in range(2):
                    pt = qkv_mm(xm, wv[c][0], wv[c][1], f"pt_v{c}_{m}")
                    nc.scalar.activation(
                        out=va_all[:, m, c * HC:(c + 1) * HC, 0:D],
                        in_=pt[:].rearrange("p (h d) -> p h d", h=HC),
                        func=AF.Identity)

        # ---- phase C: attention, two heads per pass (PE 64x128 row tiles) ----
        with contextlib.ExitStack() as pc:
            cwork = pc.enter_context(tc.tile_pool(name="cwork", bufs=1))
            for j in range(H // 2):
                ft = j
                hA, hB = 2 * j, 2 * j + 1
                pyA = ps.tile([P, TQ], f32, name=f"pyA_{j}", tag="py", bufs=2)
                pyB = ps.tile([P, TQ], f32, name=f"pyB_{j}", tag="py", bufs=2)
                for tkb in range(MKV):
                    sA = ps.tile([P, TQ], f32, name=f"sA_{j}_{tkb}", tag="scr", bufs=2)
                    sB = ps.tile([P, TQ], f32, name=f"sB_{j}_{tkb}", tag="scr", bufs=2)
                    for nk in range(TQ // NCH):
                        nsl = slice(nk * NCH, (nk + 1) * NCH)
                        nc.tensor.matmul(
                            sA[:, nsl],
                            kT_all[0:D, ft, tkb * P:(tkb + 1) * P],
                            qT_all[0:D, ft, nsl],
                            start=True, stop=True, tile_position=(0, 0))
                        nc.tensor.matmul(
                            sB[:, nsl],
                            kT_all[D:P, ft, tkb * P:(tkb + 1) * P],
                            qT_all[D:P, ft, nsl],
                            start=True, stop=True, tile_position=(64, 0))
                    pA = cwork.tile([P, TQ], bf16, name=f"pA_{j}_{tkb}", tag="p_bf", bufs=4)
                    pB = cwork.tile([P, TQ], bf16, name=f"pB_{j}_{tkb}", tag="p_bf", bufs=4)
                    nc.scalar.activation(out=pA[:], in_=sA[:], func=AF.Exp, scale=SCALE)
                    nc.scalar.activation(out=pB[:], in_=sB[:], func=AF.Exp, scale=SCALE)
                    for nk in range(TQ // NCH):
                        nsl = slice(nk * NCH, (nk + 1) * NCH)
                        nc.tensor.matmul(pyA[0:D + 1, nsl], va_all[:, tkb, hA, :],
                                         pA[:, nsl],
                                         start=(tkb == 0), stop=(tkb == MKV - 1))
                        nc.tensor.matmul(pyB[0:D + 1, nsl], va_all[:, tkb, hB, :],
                                         pB[:, nsl],
                                         start=(tkb == 0), stop=(tkb == MKV - 1))
                for name, py, r0 in (("A", pyA, 0), ("B", pyB, D)):
                    zrec = cwork.tile([P, TQ], f32, name=f"zrec{name}_{j}", tag="zrec", bufs=2)
                    nc.vector.reciprocal(out=zrec[0:1, :], in_=py[D:D + 1, :])
                    zb = dr.tile([TQ], f32, name=f"zb{name}_{j}", tag="zb")
                    nc.sync.dma_start(out=zb[:], in_=zrec[0:1, :])
                    zrep = cwork.tile([P, TQ], f32, name=f"zrep{name}_{j}", tag="zrep", bufs=2)
                    bcast_dram(zrep[0:D, :], zb[:], D)
                    nc.vector.tensor_tensor(out=yT_all[r0:r0 + D, ft, :],
                                            in0=py[0:D, :], in1=zrep[0:D, :],
                                            op=OP.mult)

        # ---- phase D: output projection ----
        with contextlib.ExitStack() as pd:
            dwork = pd.enter_context(tc.tile_pool(name="dwork", bufs=1))
            for m in range(MQ):
                po = ps.tile([P, E], f32, name=f"po_{m}", tag="scr", bufs=2)
                for nk in range(E // NCH):
                    nsl = slice(nk * NCH, (nk + 1) * NCH)
                    for kb in range(KB):
                        nc.tensor.matmul(po[:, nsl],
                                         yT_all[:, kb, m * P:(m + 1) * P],
                                         wp_all[:, kb, nsl],
                                         start=(kb == 0), stop=False)
                    nc.tensor.matmul(po[:, nsl], ones[0:1, :],
                                     bproj_row[0:1, nsl], start=False, stop=True)
                out_sb = dwork.tile([P, E], f32, tag="out_sb", bufs=3)
                nc.scalar.activation(out=out_sb[:], in_=po[:], func=AF.Identity)
                nc.sync.dma_start(out=out[m * P:(m + 1) * P, :], in_=out_sb[:])

    nc.finalize()
    return nc


def _get_nc(affine: bool):
    key = bool(affine)
    if key not in _BUILT:
        _BUILT[key] = _build_real(key)
    return _BUILT[key]


def kernel(x, Wqkv, bqkv, q_gamma, q_beta, k_gamma, k_beta, Wproj, bproj):
    from concourse.bass_utils import run_bass_kernel_spmd

    x = np.asarray(x, dtype=np.float32)
    Wqkv = np.asarray(Wqkv, dtype=np.float32)
    bqkv = np.asarray(bqkv, dtype=np.float32)
    Wproj = np.asarray(Wproj, dtype=np.float32)
    bproj = np.asarray(bproj, dtype=np.float32)
    q_gamma = np.asarray(q_gamma, dtype=np.float32)
    q_beta = np.asarray(q_beta, dtype=np.float32)
    k_gamma = np.asarray(k_gamma, dtype=np.float32)
    k_beta = np.asarray(k_beta, dtype=np.float32)

    affine = not (np.all(q_gamma == 1.0) and np.all(q_beta == 0.0)
                  and np.all(k_gamma == 1.0) and np.all(k_beta == 0.0))
    nc = _get_nc(affine)

    bf = ml_dtypes.bfloat16
    Wqkv_b = np.ascontiguousarray(Wqkv.astype(bf))
    Wproj_b = np.ascontiguousarray(Wproj.astype(bf))
    bqkv_b = bqkv.astype(bf)
    bproj_b = bproj.astype(bf)

    in_maps = []
    for c in range(NCORES):
        b, half = divmod(c, 2)
        xT_kv = np.ascontiguousarray(x[b].T.astype(bf))
        xT_q = np.ascontiguousarray(x[b, half * TQ:(half + 1) * TQ].T.astype(bf))
        in_maps.append({
            "xT_q": xT_q, "xT_kv": xT_kv,
            "Wqkv": Wqkv_b, "bqkv": bqkv_b,
            "q_gamma": q_gamma, "q_beta": q_beta,
            "k_gamma": k_gamma, "k_beta": k_beta,
            "Wproj": Wproj_b, "bproj": bproj_b,
        })

    global _last_in_maps
    _last_in_maps = in_maps
    res = run_bass_kernel_spmd(nc, in_maps, core_ids=list(range(NCORES)))
    y = np.empty((B, T, E), dtype=np.float32)
    for c in range(NCORES):
        b, half = divmod(c, 2)
        y[b, half * TQ:(half + 1) * TQ, :] = res.results[c]["out"]
    return y

